# revision 23
# baseline (speedup 1.0000x reference)
"""MetaFeatureExtractor Trainium2 kernel (v2: bf16 compare path).

Computes per-sample statistics over the time axis of x [B, T, C]:
  out = concat([mean, std(ddof=1), max, min, slope], axis=1) -> [B, 5C]

Sharding: pure data parallel over 8 NeuronCores (B=256 -> 32 samples/core).

Per-core layout: x_shard [32, 2048, 64] is loaded in 8 tiles of 4 samples:
  SBUF tile [128 partitions, (s=4, j=16, c=64)] where partition p holds
  T-rows [16p, 16p+16) of each sample -> 4 KiB contiguous DMA runs.

v2 engine plan (DMA envelope ~50-55us/core is the roofline):
  ACT+GPSIMD : one fp32->bf16 convert of each tile (split across the two
               engines to balance), feeding the DVE compare trees at
               2x throughput (16-bit DVE mode)
  DVE    : max / min over j via bf16 tensor_tensor trees (2x vs the fp32
           trees that were the 78us wall in v1)
  ACT    : x^2 -> f32r (PE producer), psum extraction, sqrt for std
  PE     : sum(x) via ones^T @ bitcast-f32r(x) (no copy needed, full
           rate at 256-col outputs); sum(x^2) via ones^T @ x2(f32r)
  GPSIMD : per-tile partition_all_reduce(max) over bf16 partials
Max/min are bf16-exact (monotone rounding => rel err <= 2^-9); sums are
tf32-accurate. Overall rel err ~1e-3 vs the 2e-2 gate.
"""

import threading

import numpy as np

B_TOTAL = 256
N_CORES = 8
B = B_TOTAL // N_CORES  # 32 samples per core
T = 2048
C = 64
S_PER_TILE = 4
N_TILES = B // S_PER_TILE  # 8
J = 16                      # T-rows per partition per tile
P = 128                     # partitions
OUT_COLS = 5 * C            # 320

_cache = threading.local()


def _build(
    do_endpoint=True,
    do_reduce=True,
    do_mm=True,
    do_par=True,
    do_scatter=True,
    n_tiles=N_TILES,
    rep=1,
    loop_n=0,
    gp_tiles=(),  # GPSIMD software elementwise is ~10x slower than cost model
    sq_dve_tiles=(1, 3, 5, 6, 7),  # tiles whose square runs on DVE (bf16)
    fused_dma=True,
    use_bitcast=False,  # neuronxcc rejects f32r-bitcast matmul rhs
    ar_bf16_in=True,
    ar_tail=False,  # single whole-row all-reduce after the tile loop
    ps_dma=False,  # dma_start cannot read PSUM; keep ACT copies
    conv_dma_tiles=(),  # tiles converted via gpsimd SWDGE cast-DMA (SBUF->SBUF)
    dma_alt=False,  # alternate tile loads between qSP and qAct HWDGE queues
    bf16_load=False,  # load x directly as bf16 via gpsimd SWDGE cast-DMA
    sq_scatter=False,  # scatter S/Q psum rows to [4,64] blocks per tile in-loop
):
    import concourse.bacc as bacc
    import concourse.bass as bass
    import concourse.tile as tile
    from concourse import bass_isa, mybir

    f32 = mybir.dt.float32
    f32r = mybir.dt.float32r
    bf16 = mybir.dt.bfloat16
    AF = mybir.ActivationFunctionType
    Alu = mybir.AluOpType

    nc = bacc.Bacc("TRN2", target_bir_lowering=False, debug=False)

    x_ap = nc.dram_tensor("x", [B, T, C], f32, kind="ExternalInput").ap()
    y_ap = nc.dram_tensor("y", [B, OUT_COLS], f32, kind="ExternalOutput").ap()

    import contextlib

    with tile.TileContext(nc) as tc:
      for _rep in range(rep):
        loop_cm = tc.For_i(0, loop_n, 1) if loop_n else contextlib.nullcontext()
        with (
            loop_cm,
            tc.tile_pool(name="xin", bufs=3) as xpool,
            tc.tile_pool(name="xb16", bufs=3) as xbpool,
            tc.tile_pool(name="xsq", bufs=2) as x2pool,
            tc.tile_pool(name="tree", bufs=2) as tree_pool,
            tc.tile_pool(name="persist", bufs=1) as pers,
            tc.tile_pool(name="small", bufs=1) as small,
            tc.tile_pool(name="ps", bufs=4, space="PSUM") as pspool,
        ):
            # persistent accumulators / partials
            Mxb = pers.tile([P, N_TILES, S_PER_TILE, C], bf16, tag="Mxb")
            NegMnb = pers.tile([P, N_TILES, S_PER_TILE, C], bf16, tag="NegMnb")
            ARmax = pers.tile([P, N_TILES * S_PER_TILE * C], f32, tag="ARmax")
            ARmin = pers.tile([P, N_TILES * S_PER_TILE * C], f32, tag="ARmin")
            SROW = pers.tile([1, B * C], f32, tag="SROW")
            QROW = pers.tile([1, B * C], f32, tag="QROW")
            if not do_mm or n_tiles < N_TILES:
                nc.vector.memset(SROW[:], 0.0)
                nc.vector.memset(QROW[:], 0.0)
            if not do_reduce or n_tiles < N_TILES:
                nc.vector.memset(Mxb[:].rearrange("p a s c -> p (a s c)"), 0.0)
                nc.vector.memset(NegMnb[:].rearrange("p a s c -> p (a s c)"), 0.0)

            ones_f = small.tile([P, 1], f32, tag="ones_f")
            nc.vector.memset(ones_f[:], 1.0)
            ones = small.tile([P, 1], f32r, tag="ones")
            nc.scalar.copy(ones[:], ones_f[:])
            ones_b = small.tile([P, 1], bf16, tag="ones_b")
            nc.vector.memset(ones_b[:], 1.0)
            # warm the sqrt table set so the tail std-sqrt pays no table load
            sqrt_warm = small.tile([1, 1], f32, tag="sqrt_warm")
            nc.scalar.activation(sqrt_warm[:], ones_f[0:1, :], AF.Sqrt)

            OUT = small.tile([B, OUT_COLS], f32, tag="OUT")
            E = small.tile([B, 2, C], f32, tag="endpoints")
            S32 = small.tile([B, C], f32, tag="S32")
            Q32 = small.tile([B, C], f32, tag="Q32")
            TMPmin = small.tile([B, C], f32, tag="TMPmin")
            TMP1 = small.tile([B, C], f32, tag="TMP1")
            TMP2 = small.tile([B, C], f32, tag="TMP2")

            # endpoint rows for slope: x[:, 0, :] and x[:, T-1, :]
            if do_endpoint:
                nc.scalar.dma_start(out=E[:], in_=x_ap[:, 0 : T : T - 1, :])
            else:
                nc.vector.memset(E[:], 0.0)

            for i in range(n_tiles):
                load_eng = nc.scalar if (dma_alt and i % 2) else nc.sync
                src = x_ap[i * S_PER_TILE : (i + 1) * S_PER_TILE].rearrange(
                    "s (p j) c -> p s j c", p=P, j=J
                )
                xb = xbpool.tile([P, S_PER_TILE, J, C], bf16, tag="xb")
                if bf16_load:
                    # single casting DMA from HBM: fp32 read, bf16 SBUF write
                    xt = None
                    nc.gpsimd.dma_start(out=xb[:], in_=src)
                else:
                    xt = xpool.tile([P, S_PER_TILE, J, C], f32, tag="xt")
                    if fused_dma:
                        load_eng.dma_start(out=xt[:], in_=src)
                    else:
                        for s in range(S_PER_TILE):
                            ssrc = x_ap[i * S_PER_TILE + s].rearrange(
                                "(p j) c -> p j c", p=P, j=J
                            )
                            load_eng.dma_start(out=xt[:, s], in_=ssrc)

                    # one fp32 -> bf16 convert per tile
                    if i in conv_dma_tiles:
                        nc.gpsimd.dma_start(out=xb[:], in_=xt[:])
                    elif i in gp_tiles:
                        nc.gpsimd.tensor_scalar_mul(xb[:], xt[:], 1.0)
                    else:
                        nc.scalar.copy(xb[:], xt[:])

                if do_reduce:
                    # DVE: max / min over j via bf16 contiguous-block TT trees
                    Mnb = tree_pool.tile([P, S_PER_TILE, C], bf16, tag="Mnb")
                    for op, dst in ((Alu.max, Mxb[:, i]), (Alu.min, Mnb[:])):
                        tA = tree_pool.tile(
                            [P, S_PER_TILE, J // 2, C], bf16, tag="tA"
                        )
                        nc.vector.tensor_tensor(
                            out=tA[:], in0=xb[:, :, 0 : J // 2, :],
                            in1=xb[:, :, J // 2 :, :], op=op,
                        )
                        tB = tree_pool.tile(
                            [P, S_PER_TILE, J // 4, C], bf16, tag="tB"
                        )
                        nc.vector.tensor_tensor(
                            out=tB[:], in0=tA[:, :, 0 : J // 4, :],
                            in1=tA[:, :, J // 4 :, :], op=op,
                        )
                        tC = tree_pool.tile(
                            [P, S_PER_TILE, J // 8, C], bf16, tag="tC"
                        )
                        nc.vector.tensor_tensor(
                            out=tC[:], in0=tB[:, :, 0 : J // 8, :],
                            in1=tB[:, :, J // 8 :, :], op=op,
                        )
                        nc.vector.tensor_tensor(
                            out=dst, in0=tC[:, :, 0, :],
                            in1=tC[:, :, 1, :], op=op,
                        )
                    # min = -(max of negated): negate the [P, s*c] partial
                    nc.vector.tensor_scalar_mul(NegMnb[:, i], Mnb[:], -1.0)
                    if ar_bf16_in:
                        ar_in_mx = Mxb[:, i].rearrange("p s c -> p (s c)")
                        ar_in_mn = NegMnb[:, i].rearrange("p s c -> p (s c)")
                    else:
                        Mxf = tree_pool.tile([P, S_PER_TILE, C], f32, tag="Mxf")
                        Mnf = tree_pool.tile([P, S_PER_TILE, C], f32, tag="Mnf")
                        nc.vector.tensor_scalar_mul(Mxf[:], Mxb[:, i], 1.0)
                        nc.vector.tensor_scalar_mul(Mnf[:], NegMnb[:, i], 1.0)
                        ar_in_mx = Mxf[:].rearrange("p s c -> p (s c)")
                        ar_in_mn = Mnf[:].rearrange("p s c -> p (s c)")
                    if do_par and not ar_tail:
                        nc.gpsimd.partition_all_reduce(
                            out_ap=ARmax[:, bass.ts(i, S_PER_TILE * C)],
                            in_ap=ar_in_mx,
                            channels=P,
                            reduce_op=bass_isa.ReduceOp.max,
                        )
                        nc.gpsimd.partition_all_reduce(
                            out_ap=ARmin[:, bass.ts(i, S_PER_TILE * C)],
                            in_ap=ar_in_mn,
                            channels=P,
                            reduce_op=bass_isa.ReduceOp.max,
                        )

                if do_mm:
                    # squares: ACT (fp32 -> f32r) or DVE (bf16 TT mult, 2x)
                    if i in sq_dve_tiles:
                        x2b = x2pool.tile([P, S_PER_TILE, J, C], bf16, tag="x2b")
                        nc.vector.tensor_tensor(
                            out=x2b[:], in0=xb[:], in1=xb[:], op=Alu.mult
                        )
                        q_lhs, q_rhs = ones_b, x2b
                    elif bf16_load:
                        x2b = x2pool.tile([P, S_PER_TILE, J, C], bf16, tag="x2b")
                        nc.scalar.activation(x2b[:], xb[:], AF.Square)
                        q_lhs, q_rhs = ones_b, x2b
                    else:
                        x2 = x2pool.tile([P, S_PER_TILE, J, C], f32r, tag="x2")
                        nc.scalar.activation(x2[:], xt[:], AF.Square)
                        q_lhs, q_rhs = ones, x2

                    # PE: column sums accumulated over j into psum rows
                    # [1, s*c] (full rate at 256-col outputs).
                    psS = pspool.tile([1, S_PER_TILE * C], f32, tag="psS")
                    psQ = pspool.tile([1, S_PER_TILE * C], f32, tag="psQ")
                    if use_bitcast:
                        xtr = xt[:].bitcast(f32r)
                        s_lhs, s_rhs = ones, xtr
                    else:
                        s_lhs, s_rhs = ones_b, xb
                    for j in range(J):
                        nc.tensor.matmul(
                            out=psS[:],
                            lhsT=s_lhs[:],
                            rhs=s_rhs[:, :, j, :],
                            start=(j == 0),
                            stop=(j == J - 1),
                        )
                    for j in range(J):
                        nc.tensor.matmul(
                            out=psQ[:],
                            lhsT=q_lhs[:],
                            rhs=q_rhs[:, :, j, :],
                            start=(j == 0),
                            stop=(j == J - 1),
                        )
                    nc.scalar.copy(SROW[0:1, bass.ts(i, S_PER_TILE * C)], psS[:])
                    nc.scalar.copy(QROW[0:1, bass.ts(i, S_PER_TILE * C)], psQ[:])
                    if sq_scatter:
                        nc.sync.dma_start(
                            out=S32[i * S_PER_TILE : (i + 1) * S_PER_TILE, :],
                            in_=SROW[0:1, bass.ts(i, S_PER_TILE * C)].rearrange(
                                "one (s c) -> (one s) c", s=S_PER_TILE
                            ),
                        )
                        nc.sync.dma_start(
                            out=Q32[i * S_PER_TILE : (i + 1) * S_PER_TILE, :],
                            in_=QROW[0:1, bass.ts(i, S_PER_TILE * C)].rearrange(
                                "one (s c) -> (one s) c", s=S_PER_TILE
                            ),
                        )

            if do_par and ar_tail and do_reduce and n_tiles == N_TILES:
                nc.gpsimd.partition_all_reduce(
                    out_ap=ARmax[:],
                    in_ap=Mxb[:].rearrange("p a s c -> p (a s c)"),
                    channels=P,
                    reduce_op=bass_isa.ReduceOp.max,
                )
                nc.gpsimd.partition_all_reduce(
                    out_ap=ARmin[:],
                    in_ap=NegMnb[:].rearrange("p a s c -> p (a s c)"),
                    channels=P,
                    reduce_op=bass_isa.ReduceOp.max,
                )
            if not (do_par and do_reduce):
                nc.vector.memset(ARmax[:], 0.0)
                nc.vector.memset(ARmin[:], 0.0)

            # scatter rows [1, B*C] -> [B, C] tiles / output columns
            if do_scatter:
                nc.scalar.dma_start(out=OUT[:, 2 * C : 3 * C], in_=ARmax[0:1, :])
                nc.scalar.dma_start(out=TMPmin[:], in_=ARmin[0:1, :])
                if not sq_scatter:
                    nc.scalar.dma_start(out=S32[:], in_=SROW[0:1, :])
                    nc.scalar.dma_start(out=Q32[:], in_=QROW[0:1, :])
            else:
                nc.vector.memset(OUT[:, 2 * C : 3 * C], 0.0)
                nc.vector.memset(TMPmin[:], 0.0)
                nc.vector.memset(S32[:], 0.0)
                nc.vector.memset(Q32[:], 0.0)

            # min = -(max of negated)
            nc.vector.tensor_scalar_mul(OUT[:, 3 * C : 4 * C], TMPmin[:], -1.0)

            # mean = S / T
            nc.vector.tensor_scalar_mul(OUT[:, 0:C], S32[:], 1.0 / T)
            # var = (Q - S * mean) / (T - 1); std = sqrt(var)
            nc.vector.tensor_tensor(
                out=TMP1[:], in0=S32[:], in1=OUT[:, 0:C], op=Alu.mult
            )
            nc.vector.tensor_sub(TMP2[:], Q32[:], TMP1[:])
            nc.vector.tensor_scalar_mul(TMP2[:], TMP2[:], 1.0 / (T - 1))
            nc.scalar.activation(OUT[:, C : 2 * C], TMP2[:], AF.Sqrt)

            # slope = (x[:, -1, :] - x[:, 0, :]) / (T - 1)
            nc.vector.tensor_sub(TMP1[:], E[:, 1, :], E[:, 0, :])
            nc.vector.tensor_scalar_mul(OUT[:, 4 * C : 5 * C], TMP1[:], 1.0 / (T - 1))

            nc.sync.dma_start(out=y_ap, in_=OUT[:])

    nc.compile()
    return nc


def _get_nc():
    if getattr(_cache, "nc", None) is None:
        _cache.nc = _build()
    return _cache.nc


def kernel(x: np.ndarray) -> np.ndarray:
    from concourse.bass_utils import run_bass_kernel_spmd

    x = np.ascontiguousarray(x, dtype=np.float32)
    assert x.shape == (B_TOTAL, T, C), x.shape

    nc = _get_nc()
    in_maps = [{"x": x[k * B : (k + 1) * B]} for k in range(N_CORES)]
    last_err = None
    for _attempt in range(3):
        try:
            res = run_bass_kernel_spmd(nc, in_maps, list(range(N_CORES)))
            break
        except Exception as e:  # transient axon transfer errors — retry
            last_err = e
    else:
        raise last_err
    return np.concatenate([res.results[k]["y"] for k in range(N_CORES)], axis=0)


def _build_repeat(rep):
    return _build(rep=rep)


def _build_loop(n):
    return _build(loop_n=n)


# revision 29
# speedup vs baseline: 1.1190x; 1.1190x over previous
"""MetaFeatureExtractor Trainium2 kernel (v2: bf16 compare path).

Computes per-sample statistics over the time axis of x [B, T, C]:
  out = concat([mean, std(ddof=1), max, min, slope], axis=1) -> [B, 5C]

Sharding: pure data parallel over 8 NeuronCores (B=256 -> 32 samples/core).

Per-core layout: x_shard [32, 2048, 64] is loaded in 8 tiles of 4 samples:
  SBUF tile [128 partitions, (s=4, j=16, c=64)] where partition p holds
  T-rows [16p, 16p+16) of each sample -> 4 KiB contiguous DMA runs.

v2 engine plan (DMA envelope ~50-55us/core is the roofline):
  ACT+GPSIMD : one fp32->bf16 convert of each tile (split across the two
               engines to balance), feeding the DVE compare trees at
               2x throughput (16-bit DVE mode)
  DVE    : max / min over j via bf16 tensor_tensor trees (2x vs the fp32
           trees that were the 78us wall in v1)
  ACT    : x^2 -> f32r (PE producer), psum extraction, sqrt for std
  PE     : sum(x) via ones^T @ bitcast-f32r(x) (no copy needed, full
           rate at 256-col outputs); sum(x^2) via ones^T @ x2(f32r)
  GPSIMD : per-tile partition_all_reduce(max) over bf16 partials
Max/min are bf16-exact (monotone rounding => rel err <= 2^-9); sums are
tf32-accurate. Overall rel err ~1e-3 vs the 2e-2 gate.
"""

import threading

import numpy as np

B_TOTAL = 256
N_CORES = 8
B = B_TOTAL // N_CORES  # 32 samples per core
T = 2048
C = 64
S_PER_TILE = 4
N_TILES = B // S_PER_TILE  # 8
J = 16                      # T-rows per partition per tile
P = 128                     # partitions
OUT_COLS = 5 * C            # 320

_cache = threading.local()


def _build(
    do_endpoint=True,
    do_reduce=True,
    do_mm=True,
    do_par=True,
    do_scatter=True,
    n_tiles=None,
    rep=1,
    loop_n=0,
    s_per_tile=S_PER_TILE,
    gp_tiles=(),  # GPSIMD software elementwise is ~10x slower than cost model
    sq_dve_tiles=None,  # tiles whose square runs on DVE (bf16); None -> ~5/8
    fused_dma=True,
    use_bitcast=False,  # neuronxcc rejects f32r-bitcast matmul rhs
    ar_bf16_in=True,
    ar_tail=False,  # single whole-row all-reduce after the tile loop
    ps_dma=False,  # dma_start cannot read PSUM; keep ACT copies
    conv_dma_tiles=(),  # tiles converted via gpsimd SWDGE cast-DMA (SBUF->SBUF)
    dma_alt=False,  # alternate tile loads between qSP and qAct HWDGE queues
    bf16_load=False,  # load x directly as bf16 via gpsimd SWDGE cast-DMA
    sq_scatter=False,  # scatter S/Q psum rows to [4,64] blocks per tile in-loop
):
    import concourse.bacc as bacc
    import concourse.bass as bass
    import concourse.tile as tile
    from concourse import bass_isa, mybir

    S_PER_TILE = s_per_tile  # noqa: N806 — shadow module constants
    N_TILES = B // S_PER_TILE  # noqa: N806
    if n_tiles is None:
        n_tiles = N_TILES
    if sq_dve_tiles is None:
        sq_dve_tiles = tuple(i for i in range(N_TILES) if (i % 8) in (1, 3, 5, 6, 7))

    f32 = mybir.dt.float32
    f32r = mybir.dt.float32r
    bf16 = mybir.dt.bfloat16
    AF = mybir.ActivationFunctionType
    Alu = mybir.AluOpType

    nc = bacc.Bacc("TRN2", target_bir_lowering=False, debug=False)

    x_ap = nc.dram_tensor("x", [B, T, C], f32, kind="ExternalInput").ap()
    y_ap = nc.dram_tensor("y", [B, OUT_COLS], f32, kind="ExternalOutput").ap()

    import contextlib

    with tile.TileContext(nc) as tc:
      for _rep in range(rep):
        loop_cm = tc.For_i(0, loop_n, 1) if loop_n else contextlib.nullcontext()
        with (
            loop_cm,
            tc.tile_pool(name="xin", bufs=3) as xpool,
            tc.tile_pool(name="xb16", bufs=3) as xbpool,
            tc.tile_pool(name="xsq", bufs=2) as x2pool,
            tc.tile_pool(name="tree", bufs=2) as tree_pool,
            tc.tile_pool(name="persist", bufs=1) as pers,
            tc.tile_pool(name="small", bufs=1) as small,
            tc.tile_pool(name="ps", bufs=4, space="PSUM") as pspool,
        ):
            # persistent accumulators / partials
            Mxb = pers.tile([P, N_TILES, S_PER_TILE, C], bf16, tag="Mxb")
            NegMnb = pers.tile([P, N_TILES, S_PER_TILE, C], bf16, tag="NegMnb")
            ARmax = pers.tile([P, N_TILES * S_PER_TILE * C], f32, tag="ARmax")
            ARmin = pers.tile([P, N_TILES * S_PER_TILE * C], f32, tag="ARmin")
            SROW = pers.tile([1, B * C], f32, tag="SROW")
            QROW = pers.tile([1, B * C], f32, tag="QROW")
            if not do_mm or n_tiles < N_TILES:
                nc.vector.memset(SROW[:], 0.0)
                nc.vector.memset(QROW[:], 0.0)
            if not do_reduce or n_tiles < N_TILES:
                nc.vector.memset(Mxb[:].rearrange("p a s c -> p (a s c)"), 0.0)
                nc.vector.memset(NegMnb[:].rearrange("p a s c -> p (a s c)"), 0.0)

            ones_f = small.tile([P, 1], f32, tag="ones_f")
            nc.vector.memset(ones_f[:], 1.0)
            ones_b = small.tile([P, 1], bf16, tag="ones_b")
            nc.vector.memset(ones_b[:], 1.0)
            # warm the sqrt table set so the tail std-sqrt pays no table load
            sqrt_warm = small.tile([1, 1], f32, tag="sqrt_warm")
            nc.scalar.activation(sqrt_warm[:], ones_f[0:1, :], AF.Sqrt)

            OUT = small.tile([B, OUT_COLS], f32, tag="OUT")
            E = small.tile([B, 2, C], f32, tag="endpoints")
            S32 = small.tile([B, C], f32, tag="S32")
            Q32 = small.tile([B, C], f32, tag="Q32")
            TMPmin = small.tile([B, C], f32, tag="TMPmin")
            TMP1 = small.tile([B, C], f32, tag="TMP1")
            TMP2 = small.tile([B, C], f32, tag="TMP2")

            # endpoint rows for slope: x[:, 0, :] and x[:, T-1, :]
            if do_endpoint:
                nc.scalar.dma_start(out=E[:], in_=x_ap[:, 0 : T : T - 1, :])
            else:
                nc.vector.memset(E[:], 0.0)

            for i in range(n_tiles):
                load_eng = nc.scalar if (dma_alt and i % 2) else nc.sync
                src = x_ap[i * S_PER_TILE : (i + 1) * S_PER_TILE].rearrange(
                    "s (p j) c -> p s j c", p=P, j=J
                )
                xb = xbpool.tile([P, S_PER_TILE, J, C], bf16, tag="xb")
                if bf16_load:
                    # single casting DMA from HBM: fp32 read, bf16 SBUF write
                    xt = None
                    nc.gpsimd.dma_start(out=xb[:], in_=src)
                else:
                    xt = xpool.tile([P, S_PER_TILE, J, C], f32, tag="xt")
                    if fused_dma:
                        load_eng.dma_start(out=xt[:], in_=src)
                    else:
                        for s in range(S_PER_TILE):
                            ssrc = x_ap[i * S_PER_TILE + s].rearrange(
                                "(p j) c -> p j c", p=P, j=J
                            )
                            load_eng.dma_start(out=xt[:, s], in_=ssrc)

                    # one fp32 -> bf16 convert per tile
                    if i in conv_dma_tiles:
                        nc.gpsimd.dma_start(out=xb[:], in_=xt[:])
                    elif i in gp_tiles:
                        nc.gpsimd.tensor_scalar_mul(xb[:], xt[:], 1.0)
                    else:
                        nc.scalar.copy(xb[:], xt[:])

                if do_reduce:
                    # DVE: max / min over j via bf16 contiguous-block TT trees
                    Mnb = tree_pool.tile([P, S_PER_TILE, C], bf16, tag="Mnb")
                    for op, dst in ((Alu.max, Mxb[:, i]), (Alu.min, Mnb[:])):
                        tA = tree_pool.tile(
                            [P, S_PER_TILE, J // 2, C], bf16, tag="tA"
                        )
                        nc.vector.tensor_tensor(
                            out=tA[:], in0=xb[:, :, 0 : J // 2, :],
                            in1=xb[:, :, J // 2 :, :], op=op,
                        )
                        tB = tree_pool.tile(
                            [P, S_PER_TILE, J // 4, C], bf16, tag="tB"
                        )
                        nc.vector.tensor_tensor(
                            out=tB[:], in0=tA[:, :, 0 : J // 4, :],
                            in1=tA[:, :, J // 4 :, :], op=op,
                        )
                        tC = tree_pool.tile(
                            [P, S_PER_TILE, J // 8, C], bf16, tag="tC"
                        )
                        nc.vector.tensor_tensor(
                            out=tC[:], in0=tB[:, :, 0 : J // 8, :],
                            in1=tB[:, :, J // 8 :, :], op=op,
                        )
                        nc.vector.tensor_tensor(
                            out=dst, in0=tC[:, :, 0, :],
                            in1=tC[:, :, 1, :], op=op,
                        )
                    # min = -(max of negated): negate the [P, s*c] partial
                    nc.vector.tensor_scalar_mul(NegMnb[:, i], Mnb[:], -1.0)
                    if ar_bf16_in:
                        ar_in_mx = Mxb[:, i].rearrange("p s c -> p (s c)")
                        ar_in_mn = NegMnb[:, i].rearrange("p s c -> p (s c)")
                    else:
                        Mxf = tree_pool.tile([P, S_PER_TILE, C], f32, tag="Mxf")
                        Mnf = tree_pool.tile([P, S_PER_TILE, C], f32, tag="Mnf")
                        nc.vector.tensor_scalar_mul(Mxf[:], Mxb[:, i], 1.0)
                        nc.vector.tensor_scalar_mul(Mnf[:], NegMnb[:, i], 1.0)
                        ar_in_mx = Mxf[:].rearrange("p s c -> p (s c)")
                        ar_in_mn = Mnf[:].rearrange("p s c -> p (s c)")
                    if do_par and not ar_tail:
                        nc.gpsimd.partition_all_reduce(
                            out_ap=ARmax[:, bass.ts(i, S_PER_TILE * C)],
                            in_ap=ar_in_mx,
                            channels=P,
                            reduce_op=bass_isa.ReduceOp.max,
                        )
                        nc.gpsimd.partition_all_reduce(
                            out_ap=ARmin[:, bass.ts(i, S_PER_TILE * C)],
                            in_ap=ar_in_mn,
                            channels=P,
                            reduce_op=bass_isa.ReduceOp.max,
                        )

                if do_mm:
                    # squares: ACT (fp32 -> f32r) or DVE (bf16 TT mult, 2x)
                    if i in sq_dve_tiles:
                        x2b = x2pool.tile([P, S_PER_TILE, J, C], bf16, tag="x2b")
                        nc.vector.tensor_tensor(
                            out=x2b[:], in0=xb[:], in1=xb[:], op=Alu.mult
                        )
                        q_lhs, q_rhs = ones_b, x2b
                    else:
                        x2b = x2pool.tile([P, S_PER_TILE, J, C], bf16, tag="x2b")
                        nc.scalar.activation(
                            x2b[:], xb[:] if bf16_load else xt[:], AF.Square
                        )
                        q_lhs, q_rhs = ones_b, x2b

                    # PE: column sums accumulated over j into psum rows
                    # [1, s*c] (full rate at 256-col outputs).
                    psS = pspool.tile([1, S_PER_TILE * C], f32, tag="psS")
                    psQ = pspool.tile([1, S_PER_TILE * C], f32, tag="psQ")
                    if use_bitcast:
                        xtr = xt[:].bitcast(f32r)
                        s_lhs, s_rhs = ones, xtr
                    else:
                        s_lhs, s_rhs = ones_b, xb
                    for j in range(J):
                        nc.tensor.matmul(
                            out=psS[:],
                            lhsT=s_lhs[:],
                            rhs=s_rhs[:, :, j, :],
                            start=(j == 0),
                            stop=(j == J - 1),
                        )
                    for j in range(J):
                        nc.tensor.matmul(
                            out=psQ[:],
                            lhsT=q_lhs[:],
                            rhs=q_rhs[:, :, j, :],
                            start=(j == 0),
                            stop=(j == J - 1),
                        )
                    nc.scalar.copy(SROW[0:1, bass.ts(i, S_PER_TILE * C)], psS[:])
                    nc.scalar.copy(QROW[0:1, bass.ts(i, S_PER_TILE * C)], psQ[:])
                    if sq_scatter:
                        nc.sync.dma_start(
                            out=S32[i * S_PER_TILE : (i + 1) * S_PER_TILE, :],
                            in_=SROW[0:1, bass.ts(i, S_PER_TILE * C)],
                        )
                        nc.sync.dma_start(
                            out=Q32[i * S_PER_TILE : (i + 1) * S_PER_TILE, :],
                            in_=QROW[0:1, bass.ts(i, S_PER_TILE * C)],
                        )

            if do_par and ar_tail and do_reduce and n_tiles == N_TILES:
                nc.gpsimd.partition_all_reduce(
                    out_ap=ARmax[:],
                    in_ap=Mxb[:].rearrange("p a s c -> p (a s c)"),
                    channels=P,
                    reduce_op=bass_isa.ReduceOp.max,
                )
                nc.gpsimd.partition_all_reduce(
                    out_ap=ARmin[:],
                    in_ap=NegMnb[:].rearrange("p a s c -> p (a s c)"),
                    channels=P,
                    reduce_op=bass_isa.ReduceOp.max,
                )
            ar_written = (
                do_par
                and do_reduce
                and n_tiles > 0
                and (not ar_tail or n_tiles == N_TILES)
            )
            if not ar_written:
                nc.vector.memset(ARmax[:], 0.0)
                nc.vector.memset(ARmin[:], 0.0)

            # scatter rows [1, B*C] -> [B, C] tiles / output columns
            if do_scatter:
                nc.scalar.dma_start(out=OUT[:, 2 * C : 3 * C], in_=ARmax[0:1, :])
                nc.scalar.dma_start(out=TMPmin[:], in_=ARmin[0:1, :])
                if not sq_scatter:
                    nc.scalar.dma_start(out=S32[:], in_=SROW[0:1, :])
                    nc.scalar.dma_start(out=Q32[:], in_=QROW[0:1, :])
            else:
                nc.vector.memset(OUT[:, 2 * C : 3 * C], 0.0)
                nc.vector.memset(TMPmin[:], 0.0)
                nc.vector.memset(S32[:], 0.0)
                nc.vector.memset(Q32[:], 0.0)

            # min = -(max of negated)
            nc.vector.tensor_scalar_mul(OUT[:, 3 * C : 4 * C], TMPmin[:], -1.0)

            # mean = S / T
            nc.vector.tensor_scalar_mul(OUT[:, 0:C], S32[:], 1.0 / T)
            # var = (Q - S * mean) / (T - 1); std = sqrt(var)
            nc.vector.tensor_tensor(
                out=TMP1[:], in0=S32[:], in1=OUT[:, 0:C], op=Alu.mult
            )
            nc.vector.tensor_sub(TMP2[:], Q32[:], TMP1[:])
            nc.vector.tensor_scalar_mul(TMP2[:], TMP2[:], 1.0 / (T - 1))
            nc.scalar.activation(OUT[:, C : 2 * C], TMP2[:], AF.Sqrt)

            # slope = (x[:, -1, :] - x[:, 0, :]) / (T - 1)
            nc.vector.tensor_sub(TMP1[:], E[:, 1, :], E[:, 0, :])
            nc.vector.tensor_scalar_mul(OUT[:, 4 * C : 5 * C], TMP1[:], 1.0 / (T - 1))

            nc.sync.dma_start(out=y_ap, in_=OUT[:])

    nc.compile()
    return nc


def _get_nc():
    if getattr(_cache, "nc", None) is None:
        _cache.nc = _build()
    return _cache.nc


def kernel(x: np.ndarray) -> np.ndarray:
    from concourse.bass_utils import run_bass_kernel_spmd

    x = np.ascontiguousarray(x, dtype=np.float32)
    assert x.shape == (B_TOTAL, T, C), x.shape

    nc = _get_nc()
    in_maps = [{"x": x[k * B : (k + 1) * B]} for k in range(N_CORES)]
    last_err = None
    for _attempt in range(3):
        try:
            res = run_bass_kernel_spmd(nc, in_maps, list(range(N_CORES)))
            break
        except Exception as e:  # transient axon transfer errors — retry
            last_err = e
    else:
        raise last_err
    return np.concatenate([res.results[k]["y"] for k in range(N_CORES)], axis=0)


def _build_repeat(rep):
    return _build(rep=rep)


def _build_loop(n):
    return _build(loop_n=n)


# revision 31
# speedup vs baseline: 1.1407x; 1.0193x over previous
"""MetaFeatureExtractor Trainium2 kernel (v4: bf16 compute path).

Computes per-sample statistics over the time axis of x [B, T, C]:
  out = concat([mean, std(ddof=1), max, min, slope], axis=1) -> [B, 5C]

Sharding: pure data parallel over 8 NeuronCores (B=256 -> 32 samples/core).

Per-core layout: x_shard [32, 2048, 64] is loaded in 16 tiles of 2 samples:
  SBUF tile [128 partitions, (s=2, j=16, c=64)] where partition p holds
  T-rows [16p, 16p+16) of each sample -> 4 KiB contiguous DMA runs, ONE
  fused dma_start per tile (per-DMA sem overhead ~0.9us makes 4 separate
  sample DMAs ~15us slower over the kernel).

Engine plan (HW-measured; DMA envelope ~50us/core + For_i barrier/fill/
drain ~25us dominate at ~94us total):
  ACT    : fp32->bf16 convert of every tile (GPSIMD and SWDGE casts
           measured 5-10x slower than the cost model - do not use);
           square of ~3/8 of tiles (bf16 out); psum extraction; sqrt
  DVE    : max / min over j via bf16 tensor_tensor trees (2x_1p mode,
           2x fp32 - this was the 78us wall in v1); square of ~5/8 of
           tiles (bf16 TT mult); negate of min partials; epilogue math
  PE     : sum(x) = ones_b^T @ xb, sum(x^2) = ones_b^T @ x2b, all bf16
           (1 cycle/row at any width; fp32 is 4x slower, f32r needs
           >=256-col outputs)
  GPSIMD : two whole-row partition_all_reduce(max) calls in the tail
           ([128, 2048] bf16 in / f32 out; min via negated partials)
Max/min are bf16-exact (monotone rounding => rel err <= 2^-9); mean/std
bf16-accumulated in fp32 psum. Overall rel err ~1.4e-3 vs the 2e-2 gate.

Measured walls on this device (loop-timing method of test.py):
  v1 fp32 baseline 104us; bf16 trees + ACT converts 93-94us; s=2 tiles +
  tail all-reduce 93.6us. Dead ends measured: GPSIMD tensor_scalar
  convert (+250us), SWDGE cast-DMA convert (+35us), per-sample DMAs
  (+15us), dual-queue loads qSP+qAct (+10us), f32r-bitcast matmul rhs
  (neuronxcc rejects).
"""

import threading

import numpy as np

B_TOTAL = 256
N_CORES = 8
B = B_TOTAL // N_CORES  # 32 samples per core
T = 2048
C = 64
S_PER_TILE = 4
N_TILES = B // S_PER_TILE  # 8
J = 16                      # T-rows per partition per tile
P = 128                     # partitions
OUT_COLS = 5 * C            # 320

_cache = threading.local()


def _build(
    do_endpoint=True,
    do_reduce=True,
    do_mm=True,
    do_par=True,
    do_scatter=True,
    n_tiles=None,
    rep=1,
    loop_n=0,
    s_per_tile=2,
    gp_tiles=(),  # GPSIMD software elementwise is ~10x slower than cost model
    sq_dve_tiles=None,  # tiles whose square runs on DVE (bf16); None -> ~5/8
    fused_dma=True,
    use_bitcast=False,  # neuronxcc rejects f32r-bitcast matmul rhs
    ar_bf16_in=True,
    ar_tail=True,  # single whole-row all-reduce after the tile loop
    ps_dma=False,  # dma_start cannot read PSUM; keep ACT copies
    conv_dma_tiles=(),  # tiles converted via gpsimd SWDGE cast-DMA (SBUF->SBUF)
    dma_alt=False,  # alternate tile loads between qSP and qAct HWDGE queues
    bf16_load=False,  # load x directly as bf16 via gpsimd SWDGE cast-DMA
    sq_scatter=False,  # scatter S/Q psum rows to [4,64] blocks per tile in-loop
):
    import concourse.bacc as bacc
    import concourse.bass as bass
    import concourse.tile as tile
    from concourse import bass_isa, mybir

    S_PER_TILE = s_per_tile  # noqa: N806 — shadow module constants
    N_TILES = B // S_PER_TILE  # noqa: N806
    if n_tiles is None:
        n_tiles = N_TILES
    if sq_dve_tiles is None:
        sq_dve_tiles = tuple(i for i in range(N_TILES) if (i % 8) in (1, 3, 5, 6, 7))

    f32 = mybir.dt.float32
    f32r = mybir.dt.float32r
    bf16 = mybir.dt.bfloat16
    AF = mybir.ActivationFunctionType
    Alu = mybir.AluOpType

    nc = bacc.Bacc("TRN2", target_bir_lowering=False, debug=False)

    x_ap = nc.dram_tensor("x", [B, T, C], f32, kind="ExternalInput").ap()
    y_ap = nc.dram_tensor("y", [B, OUT_COLS], f32, kind="ExternalOutput").ap()

    import contextlib

    with tile.TileContext(nc) as tc:
      for _rep in range(rep):
        loop_cm = tc.For_i(0, loop_n, 1) if loop_n else contextlib.nullcontext()
        with (
            loop_cm,
            tc.tile_pool(name="xin", bufs=3) as xpool,
            tc.tile_pool(name="xb16", bufs=3) as xbpool,
            tc.tile_pool(name="xsq", bufs=2) as x2pool,
            tc.tile_pool(name="tree", bufs=2) as tree_pool,
            tc.tile_pool(name="persist", bufs=1) as pers,
            tc.tile_pool(name="small", bufs=1) as small,
            tc.tile_pool(name="ps", bufs=4, space="PSUM") as pspool,
        ):
            # persistent accumulators / partials
            Mxb = pers.tile([P, N_TILES, S_PER_TILE, C], bf16, tag="Mxb")
            NegMnb = pers.tile([P, N_TILES, S_PER_TILE, C], bf16, tag="NegMnb")
            ARmax = pers.tile([P, N_TILES * S_PER_TILE * C], f32, tag="ARmax")
            ARmin = pers.tile([P, N_TILES * S_PER_TILE * C], f32, tag="ARmin")
            SROW = pers.tile([1, B * C], f32, tag="SROW")
            QROW = pers.tile([1, B * C], f32, tag="QROW")
            if not do_mm or n_tiles < N_TILES:
                nc.vector.memset(SROW[:], 0.0)
                nc.vector.memset(QROW[:], 0.0)
            if not do_reduce or n_tiles < N_TILES:
                nc.vector.memset(Mxb[:].rearrange("p a s c -> p (a s c)"), 0.0)
                nc.vector.memset(NegMnb[:].rearrange("p a s c -> p (a s c)"), 0.0)

            ones_f = small.tile([P, 1], f32, tag="ones_f")
            nc.vector.memset(ones_f[:], 1.0)
            ones_b = small.tile([P, 1], bf16, tag="ones_b")
            nc.vector.memset(ones_b[:], 1.0)
            # warm the sqrt table set so the tail std-sqrt pays no table load
            sqrt_warm = small.tile([1, 1], f32, tag="sqrt_warm")
            nc.scalar.activation(sqrt_warm[:], ones_f[0:1, :], AF.Sqrt)

            OUT = small.tile([B, OUT_COLS], f32, tag="OUT")
            E = small.tile([B, 2, C], f32, tag="endpoints")
            S32 = small.tile([B, C], f32, tag="S32")
            Q32 = small.tile([B, C], f32, tag="Q32")
            TMPmin = small.tile([B, C], f32, tag="TMPmin")
            TMP1 = small.tile([B, C], f32, tag="TMP1")
            TMP2 = small.tile([B, C], f32, tag="TMP2")

            # endpoint rows for slope: x[:, 0, :] and x[:, T-1, :]
            if do_endpoint:
                nc.scalar.dma_start(out=E[:], in_=x_ap[:, 0 : T : T - 1, :])
            else:
                nc.vector.memset(E[:], 0.0)

            for i in range(n_tiles):
                load_eng = nc.scalar if (dma_alt and i % 2) else nc.sync
                src = x_ap[i * S_PER_TILE : (i + 1) * S_PER_TILE].rearrange(
                    "s (p j) c -> p s j c", p=P, j=J
                )
                xb = xbpool.tile([P, S_PER_TILE, J, C], bf16, tag="xb")
                if bf16_load:
                    # single casting DMA from HBM: fp32 read, bf16 SBUF write
                    xt = None
                    nc.gpsimd.dma_start(out=xb[:], in_=src)
                else:
                    xt = xpool.tile([P, S_PER_TILE, J, C], f32, tag="xt")
                    if fused_dma:
                        load_eng.dma_start(out=xt[:], in_=src)
                    else:
                        for s in range(S_PER_TILE):
                            ssrc = x_ap[i * S_PER_TILE + s].rearrange(
                                "(p j) c -> p j c", p=P, j=J
                            )
                            load_eng.dma_start(out=xt[:, s], in_=ssrc)

                    # one fp32 -> bf16 convert per tile
                    if i in conv_dma_tiles:
                        nc.gpsimd.dma_start(out=xb[:], in_=xt[:])
                    elif i in gp_tiles:
                        nc.gpsimd.tensor_scalar_mul(xb[:], xt[:], 1.0)
                    else:
                        nc.scalar.copy(xb[:], xt[:])

                if do_reduce:
                    # DVE: max / min over j via bf16 contiguous-block TT trees
                    Mnb = tree_pool.tile([P, S_PER_TILE, C], bf16, tag="Mnb")
                    for op, dst in ((Alu.max, Mxb[:, i]), (Alu.min, Mnb[:])):
                        tA = tree_pool.tile(
                            [P, S_PER_TILE, J // 2, C], bf16, tag="tA"
                        )
                        nc.vector.tensor_tensor(
                            out=tA[:], in0=xb[:, :, 0 : J // 2, :],
                            in1=xb[:, :, J // 2 :, :], op=op,
                        )
                        tB = tree_pool.tile(
                            [P, S_PER_TILE, J // 4, C], bf16, tag="tB"
                        )
                        nc.vector.tensor_tensor(
                            out=tB[:], in0=tA[:, :, 0 : J // 4, :],
                            in1=tA[:, :, J // 4 :, :], op=op,
                        )
                        tC = tree_pool.tile(
                            [P, S_PER_TILE, J // 8, C], bf16, tag="tC"
                        )
                        nc.vector.tensor_tensor(
                            out=tC[:], in0=tB[:, :, 0 : J // 8, :],
                            in1=tB[:, :, J // 8 :, :], op=op,
                        )
                        nc.vector.tensor_tensor(
                            out=dst, in0=tC[:, :, 0, :],
                            in1=tC[:, :, 1, :], op=op,
                        )
                    # min = -(max of negated): negate the [P, s*c] partial
                    nc.vector.tensor_scalar_mul(NegMnb[:, i], Mnb[:], -1.0)
                    if ar_bf16_in:
                        ar_in_mx = Mxb[:, i].rearrange("p s c -> p (s c)")
                        ar_in_mn = NegMnb[:, i].rearrange("p s c -> p (s c)")
                    else:
                        Mxf = tree_pool.tile([P, S_PER_TILE, C], f32, tag="Mxf")
                        Mnf = tree_pool.tile([P, S_PER_TILE, C], f32, tag="Mnf")
                        nc.vector.tensor_scalar_mul(Mxf[:], Mxb[:, i], 1.0)
                        nc.vector.tensor_scalar_mul(Mnf[:], NegMnb[:, i], 1.0)
                        ar_in_mx = Mxf[:].rearrange("p s c -> p (s c)")
                        ar_in_mn = Mnf[:].rearrange("p s c -> p (s c)")
                    if do_par and not ar_tail:
                        nc.gpsimd.partition_all_reduce(
                            out_ap=ARmax[:, bass.ts(i, S_PER_TILE * C)],
                            in_ap=ar_in_mx,
                            channels=P,
                            reduce_op=bass_isa.ReduceOp.max,
                        )
                        nc.gpsimd.partition_all_reduce(
                            out_ap=ARmin[:, bass.ts(i, S_PER_TILE * C)],
                            in_ap=ar_in_mn,
                            channels=P,
                            reduce_op=bass_isa.ReduceOp.max,
                        )

                if do_mm:
                    # squares: ACT (fp32 -> f32r) or DVE (bf16 TT mult, 2x)
                    if i in sq_dve_tiles:
                        x2b = x2pool.tile([P, S_PER_TILE, J, C], bf16, tag="x2b")
                        nc.vector.tensor_tensor(
                            out=x2b[:], in0=xb[:], in1=xb[:], op=Alu.mult
                        )
                        q_lhs, q_rhs = ones_b, x2b
                    else:
                        x2b = x2pool.tile([P, S_PER_TILE, J, C], bf16, tag="x2b")
                        nc.scalar.activation(
                            x2b[:], xb[:] if bf16_load else xt[:], AF.Square
                        )
                        q_lhs, q_rhs = ones_b, x2b

                    # PE: column sums accumulated over j into psum rows
                    # [1, s*c] (full rate at 256-col outputs).
                    psS = pspool.tile([1, S_PER_TILE * C], f32, tag="psS")
                    psQ = pspool.tile([1, S_PER_TILE * C], f32, tag="psQ")
                    if use_bitcast:
                        xtr = xt[:].bitcast(f32r)
                        s_lhs, s_rhs = ones, xtr
                    else:
                        s_lhs, s_rhs = ones_b, xb
                    for j in range(J):
                        nc.tensor.matmul(
                            out=psS[:],
                            lhsT=s_lhs[:],
                            rhs=s_rhs[:, :, j, :],
                            start=(j == 0),
                            stop=(j == J - 1),
                        )
                    for j in range(J):
                        nc.tensor.matmul(
                            out=psQ[:],
                            lhsT=q_lhs[:],
                            rhs=q_rhs[:, :, j, :],
                            start=(j == 0),
                            stop=(j == J - 1),
                        )
                    nc.scalar.copy(SROW[0:1, bass.ts(i, S_PER_TILE * C)], psS[:])
                    nc.scalar.copy(QROW[0:1, bass.ts(i, S_PER_TILE * C)], psQ[:])
                    if sq_scatter:
                        nc.sync.dma_start(
                            out=S32[i * S_PER_TILE : (i + 1) * S_PER_TILE, :],
                            in_=SROW[0:1, bass.ts(i, S_PER_TILE * C)],
                        )
                        nc.sync.dma_start(
                            out=Q32[i * S_PER_TILE : (i + 1) * S_PER_TILE, :],
                            in_=QROW[0:1, bass.ts(i, S_PER_TILE * C)],
                        )

            if do_par and ar_tail and do_reduce and n_tiles == N_TILES:
                nc.gpsimd.partition_all_reduce(
                    out_ap=ARmax[:],
                    in_ap=Mxb[:].rearrange("p a s c -> p (a s c)"),
                    channels=P,
                    reduce_op=bass_isa.ReduceOp.max,
                )
                nc.gpsimd.partition_all_reduce(
                    out_ap=ARmin[:],
                    in_ap=NegMnb[:].rearrange("p a s c -> p (a s c)"),
                    channels=P,
                    reduce_op=bass_isa.ReduceOp.max,
                )
            ar_written = (
                do_par
                and do_reduce
                and n_tiles > 0
                and (not ar_tail or n_tiles == N_TILES)
            )
            if not ar_written:
                nc.vector.memset(ARmax[:], 0.0)
                nc.vector.memset(ARmin[:], 0.0)

            # scatter rows [1, B*C] -> [B, C] tiles / output columns
            if do_scatter:
                nc.scalar.dma_start(out=OUT[:, 2 * C : 3 * C], in_=ARmax[0:1, :])
                nc.scalar.dma_start(out=TMPmin[:], in_=ARmin[0:1, :])
                if not sq_scatter:
                    nc.scalar.dma_start(out=S32[:], in_=SROW[0:1, :])
                    nc.scalar.dma_start(out=Q32[:], in_=QROW[0:1, :])
            else:
                nc.vector.memset(OUT[:, 2 * C : 3 * C], 0.0)
                nc.vector.memset(TMPmin[:], 0.0)
                nc.vector.memset(S32[:], 0.0)
                nc.vector.memset(Q32[:], 0.0)

            # min = -(max of negated)
            nc.vector.tensor_scalar_mul(OUT[:, 3 * C : 4 * C], TMPmin[:], -1.0)

            # mean = S / T
            nc.vector.tensor_scalar_mul(OUT[:, 0:C], S32[:], 1.0 / T)
            # var = (Q - S * mean) / (T - 1); std = sqrt(var)
            nc.vector.tensor_tensor(
                out=TMP1[:], in0=S32[:], in1=OUT[:, 0:C], op=Alu.mult
            )
            nc.vector.tensor_sub(TMP2[:], Q32[:], TMP1[:])
            nc.vector.tensor_scalar_mul(TMP2[:], TMP2[:], 1.0 / (T - 1))
            nc.scalar.activation(OUT[:, C : 2 * C], TMP2[:], AF.Sqrt)

            # slope = (x[:, -1, :] - x[:, 0, :]) / (T - 1)
            nc.vector.tensor_sub(TMP1[:], E[:, 1, :], E[:, 0, :])
            nc.vector.tensor_scalar_mul(OUT[:, 4 * C : 5 * C], TMP1[:], 1.0 / (T - 1))

            nc.sync.dma_start(out=y_ap, in_=OUT[:])

    nc.compile()
    return nc


def _get_nc():
    if getattr(_cache, "nc", None) is None:
        _cache.nc = _build()
    return _cache.nc


def kernel(x: np.ndarray) -> np.ndarray:
    from concourse.bass_utils import run_bass_kernel_spmd

    x = np.ascontiguousarray(x, dtype=np.float32)
    assert x.shape == (B_TOTAL, T, C), x.shape

    nc = _get_nc()
    in_maps = [{"x": x[k * B : (k + 1) * B]} for k in range(N_CORES)]
    last_err = None
    for _attempt in range(3):
        try:
            res = run_bass_kernel_spmd(nc, in_maps, list(range(N_CORES)))
            break
        except Exception as e:  # transient axon transfer errors — retry
            last_err = e
    else:
        raise last_err
    return np.concatenate([res.results[k]["y"] for k in range(N_CORES)], axis=0)


def _build_repeat(rep):
    return _build(rep=rep)


def _build_loop(n):
    return _build(loop_n=n)
